# revision 3
# baseline (speedup 1.0000x reference)
"""MCR2 (Maximal Coding Rate Reduction) loss kernel for 8 Trainium2 NeuronCores.

Strategy
--------
The loss is built from (k+1) tiny 64x64 Gram matrices reduced over m=262144
samples: G_total = E^T E and per-class G_j = E_j^T E_j (classes partition the
sample set), followed by slogdet on the 64x64 matrices.

Sharding: data-parallel over the sample axis. On the host we sort samples by
class (a Gram is permutation-invariant), pad each class block with zero rows
(zeros contribute nothing to a Gram) so every device gets an identical number
of 128-row class-pure chunks, and pre-pack each device shard partition-major
so the device DMA is fully contiguous. Each core accumulates its class Grams
with TensorEngine matmuls (lhsT = rhs = chunk -> chunk^T @ chunk) into fixed
PSUM slots, using two concurrent 64-column PE tile groups (tile_position
(0,0)/(0,64)) so the 128x128 array is fully used despite p=64. The 8 partial
[128, 640] Gram blocks are summed on the host, where the 11 slogdets of
64x64 matrices (~3 MFLOP total, vs 2.1 GFLOP of Gram work on device) and the
final scalar combine run in float64.

Inputs are rounded to bfloat16 for the device matmuls: the systematic Gram
perturbation cancels between the discriminative and compressive terms, so
the end-to-end loss matches the fp32 reference to ~1e-4 relative (measured),
while halving DMA bytes.
"""

import numpy as np
import ml_dtypes

NCORES = 8
P = 64  # feature dim
NCLASS = 10
CHUNK = 128
GAM1 = 1.0
GAM2 = 1.0
EPS = 0.01

COMPUTE_DTYPE = "bfloat16"  # "float32" | "bfloat16" | "float8e4"
DUAL_STREAM = True  # use two concurrent 64-col PE tile groups
G_DMA = 30  # chunks per input DMA

PROFILE = False  # set True (e.g. from test.py) to capture NTFF timing
LAST_EXEC_NS = None
LAST_RESULTS = None

_NP_DT = {
    "float32": np.float32,
    "bfloat16": ml_dtypes.bfloat16,
    "float8e4": ml_dtypes.float8_e4m3,
}

_prog_cache = {}


def _build_program(chunks_dev, dt_name, dual):
    """Build + compile the SPMD single-core program.

    chunks_dev: per-class chunk count on each device (identical across cores).
    Returns (nc, meta) where meta carries the (stream, class) presence mask.
    """
    import concourse.bass as bass
    import concourse.bacc as bacc
    import concourse.mybir as mybir
    import concourse.tile as tile

    C = int(sum(chunks_dev))
    dt = getattr(mybir.dt, dt_name)
    out_parts = 128 if dual else 64

    nc = bacc.Bacc("TRN2", target_bir_lowering=False, debug=False,
                   num_devices=NCORES)
    x = nc.dram_tensor("x", [CHUNK, C * P], dt, kind="ExternalInput")
    out_d = nc.dram_tensor("out", [out_parts, NCLASS * P], mybir.dt.float32,
                           kind="ExternalOutput")

    # chunk -> class map and chunk -> stream assignment (alternate within a
    # class block so both streams see every class unless the block is tiny)
    classes = []
    for j, n in enumerate(chunks_dev):
        classes += [j] * int(n)
    streams = []
    percls = [0] * NCLASS
    for c in range(C):
        j = classes[c]
        streams.append(percls[j] % 2 if dual else 0)
        percls[j] += 1

    # per (stream, class): total MMs, for start/stop flags
    totals = {}
    for c in range(C):
        key = (streams[c], classes[c])
        totals[key] = totals.get(key, 0) + 1
    seen = {k: 0 for k in totals}

    with tile.TileContext(nc) as tc:
        with (
            tc.tile_pool(name="inp", bufs=4) as pool,
            tc.tile_pool(name="ps", bufs=1, space=bass.MemorySpace.PSUM) as pps,
            tc.tile_pool(name="res", bufs=1) as pres,
        ):
            psum = pps.tile([out_parts, NCLASS * P], mybir.dt.float32)
            for g0 in range(0, C, G_DMA):
                gn = min(G_DMA, C - g0)
                t = pool.tile([CHUNK, gn * P], dt, tag="in")
                nc.sync.dma_start(t[:], x[:, g0 * P:(g0 + gn) * P])
                for c in range(g0, g0 + gn):
                    j, s = classes[c], streams[c]
                    seen[(s, j)] += 1
                    sl = psum[s * P:(s + 1) * P, j * P:(j + 1) * P]
                    lhs = t[:, (c - g0) * P:(c - g0 + 1) * P]
                    nc.tensor.matmul(
                        sl, lhs, lhs,
                        start=(seen[(s, j)] == 1),
                        stop=(seen[(s, j)] == totals[(s, j)]),
                        tile_position=(0, s * P),
                    )
            res = pres.tile([out_parts, NCLASS * P], mybir.dt.float32)
            nc.vector.tensor_copy(res[:], psum[:])
            nc.sync.dma_start(out_d[:], res[:])

    nc.compile()
    meta = {"present": totals, "C": C, "dual": dual}
    return nc, meta


def _pack_shards(embed, targets):
    """Sort by class, split per class across cores, zero-pad to class-pure
    128-row chunks, and pack each shard partition-major ([128, C*64])."""
    m = embed.shape[0]
    t = np.asarray(targets).astype(np.int64).ravel()
    counts = np.bincount(t, minlength=NCLASS).astype(np.int64)
    order = np.argsort(t, kind="stable")
    se = np.ascontiguousarray(np.asarray(embed, dtype=np.float32)[order])

    chunks_dev = np.maximum(1, -(-counts // (NCORES * CHUNK))).astype(int)
    C = int(chunks_dev.sum())
    X = np.zeros((NCORES, C * CHUNK, P), dtype=np.float32)
    cls_ofs = np.concatenate([[0], np.cumsum(counts)])
    row0 = np.concatenate([[0], np.cumsum(chunks_dev * CHUNK)])
    for j in range(NCLASS):
        cj = int(counts[j])
        base, rem = divmod(cj, NCORES)
        sizes = base + (np.arange(NCORES) < rem)
        starts = cls_ofs[j] + np.concatenate([[0], np.cumsum(sizes)[:-1]])
        assert sizes.max() <= chunks_dev[j] * CHUNK
        for d in range(NCORES):
            X[d, row0[j]:row0[j] + sizes[d]] = se[starts[d]:starts[d] + sizes[d]]

    Xc = X.astype(_NP_DT[COMPUTE_DTYPE])
    packed = np.ascontiguousarray(
        Xc.reshape(NCORES, C, CHUNK, P).transpose(0, 2, 1, 3)
        .reshape(NCORES, CHUNK, C * P))
    return packed, counts, tuple(int(v) for v in chunks_dev), m


def _ensure_ntff_hook():
    """The agent image's antenv lacks axon_hooks; synthesize it and register
    the ctypes NTFF profile hook so run_bass_kernel_spmd(trace=True) works."""
    import sys, types
    try:
        import antenv.axon_hooks  # noqa: F401
        return True
    except ImportError:
        pass
    try:
        import antenv
        from trn_agent_boot.trn_boot import _ntff_profile_via_ctypes
        mod = types.ModuleType("antenv.axon_hooks")
        _hook = [None]
        mod.set_axon_ntff_profile_hook = lambda h: _hook.__setitem__(0, h)
        mod.get_axon_ntff_profile_hook = lambda: _hook[0]
        sys.modules["antenv.axon_hooks"] = mod
        antenv.axon_hooks = mod
        mod.set_axon_ntff_profile_hook(
            _ntff_profile_via_ctypes("/opt/axon/libaxon_pjrt.so"))
        return True
    except Exception:
        return False


def kernel(embed, targets):
    global LAST_EXEC_NS, LAST_RESULTS
    packed, counts, chunks_dev, m = _pack_shards(embed, targets)

    key = (chunks_dev, COMPUTE_DTYPE, DUAL_STREAM, G_DMA)
    if key not in _prog_cache:
        _prog_cache[key] = _build_program(chunks_dev, COMPUTE_DTYPE, DUAL_STREAM)
    nc, meta = _prog_cache[key]

    from concourse.bass_utils import run_bass_kernel_spmd
    in_maps = [{"x": packed[d]} for d in range(NCORES)]
    do_trace = bool(PROFILE) and _ensure_ntff_hook()
    res = run_bass_kernel_spmd(nc, in_maps, core_ids=list(range(NCORES)),
                               trace=do_trace)
    LAST_EXEC_NS = res.exec_time_ns
    LAST_RESULTS = res

    # host reduction: sum partial Grams over cores (and over both PE streams)
    grams = np.zeros((NCLASS, P, P), dtype=np.float64)
    for r in res.results:
        o = np.asarray(r["out"], dtype=np.float64)
        for (s, j) in meta["present"]:
            grams[j] += o[s * P:(s + 1) * P, j * P:(j + 1) * P]

    eye = np.eye(P, dtype=np.float64)
    g_tot = grams.sum(axis=0)
    ld_tot = np.linalg.slogdet(eye + GAM1 * (P / (m * EPS)) * g_tot)[1]
    tr_pi = counts.astype(np.float64) + 1e-8
    compress = 0.0
    for j in range(NCLASS):
        ldj = np.linalg.slogdet(eye + (P / (tr_pi[j] * EPS)) * grams[j])[1]
        compress += ldj * tr_pi[j] / m / 2.0
    loss = GAM2 * (-ld_tot / 2.0) + compress
    return np.array(loss, dtype=np.float32)


# revision 4
# speedup vs baseline: 1.0609x; 1.0609x over previous
"""MCR2 (Maximal Coding Rate Reduction) loss kernel for 8 Trainium2 NeuronCores.

Strategy
--------
The loss is built from (k+1) tiny 64x64 Gram matrices reduced over m=262144
samples: G_total = E^T E and per-class G_j = E_j^T E_j (classes partition the
sample set, so G_total = sum_j G_j), followed by slogdet on 64x64 matrices.

Sharding: data-parallel over the sample axis. On the host we sort samples by
class (a Gram is permutation-invariant), pad each class block with zero rows
(zeros contribute nothing to a Gram) so every device gets an identical even
number of 128-row class-pure chunks, and pre-pack each device shard
partition-major so the device DMA is fully contiguous.

Device compute: chunks are processed in same-class PAIRS. For a pair [A|B]
(SBUF tile [128, 128]) a single self-loading matmul [A|B]^T @ [A|B]
accumulates into a per-class PSUM block [128, 128] whose diagonal 64x64
blocks are A^T A and B^T B — the off-diagonal cross terms are ignored on the
host. This keeps the full 128x128 PE array busy (p=64 would otherwise idle
half the columns), halves the LDWEIGHTS serial cost per sample (128-col bf16
weight loads are FWL-eligible), and halves PE instruction count.

The 8 partial [128, 1280] PSUM images are summed on the host, where the 11
slogdets of 64x64 matrices (~3 MFLOP, vs ~2.1 GFLOP of Gram work on device)
and the final scalar combine run in float64.

Inputs are rounded to bfloat16 for the device matmuls: the systematic Gram
perturbation cancels between the discriminative and compressive terms, so
the end-to-end loss matches the fp32 reference to ~1.3e-4 relative
(measured; the fp32 reference itself sits ~2e-4 from the float64 truth),
while halving DMA bytes.
"""

import numpy as np
import ml_dtypes

NCORES = 8
P = 64  # feature dim
NCLASS = 10
CHUNK = 128
GAM1 = 1.0
GAM2 = 1.0
EPS = 0.01

COMPUTE_DTYPE = "bfloat16"  # "bfloat16" | "float8e4"
PSUM_DMA_DIRECT = False  # DMA straight from PSUM to DRAM (skip DVE copy)

PROFILE = False  # set True (e.g. from test.py) to capture NTFF timing
LAST_EXEC_NS = None
LAST_RESULTS = None

_NP_DT = {
    "float32": np.float32,
    "bfloat16": ml_dtypes.bfloat16,
    "float8e4": ml_dtypes.float8_e4m3,
}

_prog_cache = {}


def _group_plan(C):
    """DMA group sizes (in chunks, all even so pairs never straddle a DMA):
    small leading groups so the PE starts early, then large batched ones."""
    plan = []
    left = C
    for want in (8, 8, 16):
        if left <= 0:
            break
        g = min(want, left)
        if g % 2:
            g += 1
        plan.append(g)
        left -= g
    while left > 0:
        g = min(32, left)
        plan.append(g)
        left -= g
    return plan


def _build_program(chunks_dev, dt_name):
    """Build + compile the per-core program (identical across cores)."""
    import concourse.bass as bass
    import concourse.bacc as bacc
    import concourse.mybir as mybir
    import concourse.tile as tile

    C = int(sum(chunks_dev))
    assert C % 2 == 0 and all(n % 2 == 0 for n in chunks_dev)
    dt = getattr(mybir.dt, dt_name)

    nc = bacc.Bacc("TRN2", target_bir_lowering=False, debug=False,
                   num_devices=NCORES)
    x = nc.dram_tensor("x", [CHUNK, C * P], dt, kind="ExternalInput")
    out_d = nc.dram_tensor("out", [CHUNK, NCLASS * CHUNK], mybir.dt.float32,
                           kind="ExternalOutput")

    # chunk -> class map (class blocks are contiguous, even-sized)
    classes = []
    for j, n in enumerate(chunks_dev):
        classes += [j] * int(n)

    pairs_total = [int(n) // 2 for n in chunks_dev]
    pair_seen = [0] * NCLASS

    groups = _group_plan(C)
    with tile.TileContext(nc) as tc:
        with (
            tc.tile_pool(name="inp", bufs=len(groups)) as pool,
            tc.tile_pool(name="ps", bufs=1, space=bass.MemorySpace.PSUM) as pps,
            tc.tile_pool(name="res", bufs=1) as pres,
        ):
            psum = pps.tile([CHUNK, NCLASS * CHUNK], mybir.dt.float32)
            g0 = 0
            for gn in groups:
                t = pool.tile([CHUNK, gn * P], dt, tag="in")
                nc.sync.dma_start(t[:], x[:, g0 * P:(g0 + gn) * P])
                for c in range(g0, g0 + gn, 2):
                    j = classes[c]
                    assert classes[c + 1] == j
                    pair_seen[j] += 1
                    sl = psum[:, j * CHUNK:(j + 1) * CHUNK]
                    ab = t[:, (c - g0) * P:(c - g0 + 2) * P]
                    nc.tensor.matmul(
                        sl, ab, ab,
                        start=(pair_seen[j] == 1),
                        stop=(pair_seen[j] == pairs_total[j]),
                    )
                g0 += gn
            if PSUM_DMA_DIRECT:
                nc.sync.dma_start(out_d[:], psum[:])
            else:
                res = pres.tile([CHUNK, NCLASS * CHUNK], mybir.dt.float32)
                nc.vector.tensor_copy(res[:], psum[:])
                nc.sync.dma_start(out_d[:], res[:])

    nc.compile()
    return nc, {"C": C}


def _pack_shards(embed, targets):
    """Sort by class, split per class across cores, zero-pad to an even
    number of class-pure 128-row chunks per core, pack partition-major."""
    m = embed.shape[0]
    t = np.asarray(targets).astype(np.int64).ravel()
    counts = np.bincount(t, minlength=NCLASS).astype(np.int64)
    order = np.argsort(t, kind="stable")
    se = np.ascontiguousarray(np.asarray(embed, dtype=np.float32)[order])

    # even chunk count per class per device
    chunks_dev = 2 * np.maximum(1, -(-counts // (NCORES * 2 * CHUNK))).astype(int)
    C = int(chunks_dev.sum())
    X = np.zeros((NCORES, C * CHUNK, P), dtype=np.float32)
    cls_ofs = np.concatenate([[0], np.cumsum(counts)])
    row0 = np.concatenate([[0], np.cumsum(chunks_dev * CHUNK)])
    for j in range(NCLASS):
        cj = int(counts[j])
        base, rem = divmod(cj, NCORES)
        sizes = base + (np.arange(NCORES) < rem)
        starts = cls_ofs[j] + np.concatenate([[0], np.cumsum(sizes)[:-1]])
        assert sizes.max() <= chunks_dev[j] * CHUNK
        for d in range(NCORES):
            X[d, row0[j]:row0[j] + sizes[d]] = se[starts[d]:starts[d] + sizes[d]]

    Xc = X.astype(_NP_DT[COMPUTE_DTYPE])
    packed = np.ascontiguousarray(
        Xc.reshape(NCORES, C, CHUNK, P).transpose(0, 2, 1, 3)
        .reshape(NCORES, CHUNK, C * P))
    return packed, counts, tuple(int(v) for v in chunks_dev), m


def _ensure_ntff_hook():
    """The agent image's antenv lacks axon_hooks; synthesize it and register
    the ctypes NTFF profile hook so run_bass_kernel_spmd(trace=True) works."""
    import sys, types
    try:
        import antenv.axon_hooks  # noqa: F401
        return True
    except ImportError:
        pass
    try:
        import antenv
        from trn_agent_boot.trn_boot import _ntff_profile_via_ctypes
        mod = types.ModuleType("antenv.axon_hooks")
        _hook = [None]
        mod.set_axon_ntff_profile_hook = lambda h: _hook.__setitem__(0, h)
        mod.get_axon_ntff_profile_hook = lambda: _hook[0]
        sys.modules["antenv.axon_hooks"] = mod
        antenv.axon_hooks = mod
        mod.set_axon_ntff_profile_hook(
            _ntff_profile_via_ctypes("/opt/axon/libaxon_pjrt.so"))
        return True
    except Exception:
        return False


def kernel(embed, targets):
    global LAST_EXEC_NS, LAST_RESULTS
    packed, counts, chunks_dev, m = _pack_shards(embed, targets)

    key = (chunks_dev, COMPUTE_DTYPE, PSUM_DMA_DIRECT)
    if key not in _prog_cache:
        _prog_cache[key] = _build_program(chunks_dev, COMPUTE_DTYPE)
    nc, meta = _prog_cache[key]

    from concourse.bass_utils import run_bass_kernel_spmd
    in_maps = [{"x": packed[d]} for d in range(NCORES)]
    do_trace = bool(PROFILE) and _ensure_ntff_hook()
    res = run_bass_kernel_spmd(nc, in_maps, core_ids=list(range(NCORES)),
                               trace=do_trace)
    LAST_EXEC_NS = res.exec_time_ns
    LAST_RESULTS = res

    # host reduction: per-class Gram = sum over cores of the two diagonal
    # 64x64 blocks of that class's [128, 128] PSUM image
    grams = np.zeros((NCLASS, P, P), dtype=np.float64)
    for r in res.results:
        o = np.asarray(r["out"], dtype=np.float64)
        for j in range(NCLASS):
            blk = o[:, j * CHUNK:(j + 1) * CHUNK]
            grams[j] += blk[:P, :P] + blk[P:, P:]

    eye = np.eye(P, dtype=np.float64)
    g_tot = grams.sum(axis=0)
    ld_tot = np.linalg.slogdet(eye + GAM1 * (P / (m * EPS)) * g_tot)[1]
    tr_pi = counts.astype(np.float64) + 1e-8
    compress = 0.0
    for j in range(NCLASS):
        ldj = np.linalg.slogdet(eye + (P / (tr_pi[j] * EPS)) * grams[j])[1]
        compress += ldj * tr_pi[j] / m / 2.0
    loss = GAM2 * (-ld_tot / 2.0) + compress
    return np.array(loss, dtype=np.float32)


# revision 7
# speedup vs baseline: 1.1906x; 1.1222x over previous
"""MCR2 (Maximal Coding Rate Reduction) loss kernel for 8 Trainium2 NeuronCores.

Strategy
--------
The loss is built from (k+1) tiny 64x64 Gram matrices reduced over m=262144
samples: G_total = E^T E and per-class G_j = E_j^T E_j (classes partition the
sample set, so G_total = sum_j G_j), followed by slogdet on 64x64 matrices.

Sharding: data-parallel over the sample axis. On the host we sort samples by
class (a Gram is permutation-invariant), pad each class block with zero rows
(zeros contribute nothing to a Gram) so every device gets an identical even
number of 128-row class-pure chunks, and pre-pack each device shard
partition-major so the device DMA is fully contiguous.

Device compute (raw bass, no Tile): chunks are processed in same-class PAIRS.
For a pair [A|B] (SBUF tile [128, 128]) a single self-loading matmul
[A|B]^T @ [A|B] accumulates into a per-class PSUM block [128, 128] whose
diagonal 64x64 blocks are A^T A and B^T B — the off-diagonal cross terms are
never read back. This keeps the full 128x128 PE array busy (p=64 would
otherwise idle half the columns) and halves PE instruction count. Raw bass is
used instead of Tile because Tile's legalizer splits matmuls into standalone
LDWEIGHTS whose issue never reaches the warm 2.4 GHz clock rate in this
kernel shape; the fused self-loading matmul stream measures ~56ns/pair warm
vs ~107ns via Tile. A short burst of scratch warm-up matmuls runs during the
initial DMA fill so the PE HAM clock gate is already released when real data
arrives. The whole shard stays resident in SBUF (~35KB/partition) so the PE
never waits on buffer recycling.

The 8 partial Gram images are summed on the host, where the 11 slogdets of
64x64 matrices (~3 MFLOP, vs ~2.1 GFLOP of Gram work on device) and the
final scalar combine run in float64.

Inputs are rounded to bfloat16 for the device matmuls: the systematic Gram
perturbation cancels between the discriminative and compressive terms, so
the end-to-end loss matches the fp32 reference to ~1.3e-4 relative
(measured; the fp32 reference itself sits ~2e-4 from the float64 truth),
while halving DMA bytes.
"""

import numpy as np
import ml_dtypes

NCORES = 8
P = 64  # feature dim
NCLASS = 10
CHUNK = 128
GAM1 = 1.0
GAM2 = 1.0
EPS = 0.01

COMPUTE_DTYPE = "bfloat16"  # "bfloat16" | "float8e4"
NWARM = 14  # scratch matmuls issued during the DMA fill to warm the PE clock

PROFILE = False  # set True (e.g. from test.py) to capture NTFF timing
LAST_EXEC_NS = None
LAST_RESULTS = None

_NP_DT = {
    "float32": np.float32,
    "bfloat16": ml_dtypes.bfloat16,
    "float8e4": ml_dtypes.float8_e4m3,
}

_prog_cache = {}


def _group_plan(C):
    """DMA group sizes in chunks (all even so pairs never straddle a DMA):
    small leading groups so the PE starts early, then large batched ones."""
    plan = []
    left = C
    for want in (8, 8, 16):
        if left <= 0:
            break
        g = min(want, left)
        if g % 2:
            g += 1
        plan.append(g)
        left -= g
    while left > 0:
        g = min(32, left)
        plan.append(g)
        left -= g
    return plan


def _build_program(chunks_dev, dt_name):
    """Build + compile the per-core raw-bass program (identical across cores)."""
    import concourse.bacc as bacc
    import concourse.mybir as mybir

    C = int(sum(chunks_dev))
    assert C % 2 == 0 and all(n % 2 == 0 for n in chunks_dev)
    dt = getattr(mybir.dt, dt_name)
    f32 = mybir.dt.float32

    nc = bacc.Bacc("TRN2", target_bir_lowering=False, debug=False,
                   num_devices=NCORES)
    x = nc.dram_tensor("x", [CHUNK, C * P], dt, kind="ExternalInput")
    out_d = nc.dram_tensor("out", [CHUNK, NCLASS * P], f32,
                           kind="ExternalOutput")

    classes = []
    for j, n in enumerate(chunks_dev):
        classes += [j] * int(n)
    pairs_total = [int(n) // 2 for n in chunks_dev]
    pair_seen = [0] * NCLASS
    groups = _group_plan(C)

    from contextlib import ExitStack
    with ExitStack() as stack:
        t = stack.enter_context(nc.sbuf_tensor([CHUNK, C * P], dt))
        # never written: garbage contents are fine, it only warms the PE clock
        warm_t = stack.enter_context(nc.sbuf_tensor([CHUNK, CHUNK], dt))
        ps = stack.enter_context(
            nc.psum_tensor([CHUNK, NCLASS * CHUNK + CHUNK], f32))
        r = stack.enter_context(nc.sbuf_tensor([CHUNK, NCLASS * P], f32))
        # one semaphore per input DMA: the 16 per-engine slice completions of
        # different DMAs are not FIFO across groups, so a single counting
        # semaphore would let group gi's matmuls run on slices of LATER groups
        grp_sem = [stack.enter_context(nc.semaphore(f"grp_sem_{gi}"))
                   for gi in range(len(groups))]
        pe_sem = stack.enter_context(nc.semaphore())
        dve_sem = stack.enter_context(nc.semaphore())
        block = stack.enter_context(nc.Block())

        scratch = ps[:, NCLASS * CHUNK:NCLASS * CHUNK + CHUNK]

        @block.sync
        def _(sync):
            g0 = 0
            for gi, gn in enumerate(groups):
                sync.dma_start(
                    t[:, g0 * P:(g0 + gn) * P],
                    x[:, g0 * P:(g0 + gn) * P],
                ).then_inc(grp_sem[gi], 16)
                g0 += gn
            sync.wait_ge(dve_sem, 1)
            sync.dma_start(out_d[:], r[:]).then_inc(pe_sem, 16)

        @block.tensor
        def _(tensor):
            for _ in range(NWARM):
                nc.tensor.matmul(scratch, warm_t[:], warm_t[:],
                                 start=True, stop=True)
            g0 = 0
            mm = None
            for gi, gn in enumerate(groups):
                tensor.wait_ge(grp_sem[gi], 16)
                for c in range(g0, g0 + gn, 2):
                    j = classes[c]
                    pair_seen[j] += 1
                    sl = t[:, c * P:(c + 2) * P]
                    mm = nc.tensor.matmul(
                        ps[:, j * CHUNK:(j + 1) * CHUNK], sl, sl,
                        start=(pair_seen[j] == 1),
                        stop=(pair_seen[j] == pairs_total[j]),
                    )
                g0 += gn
            mm.then_inc(pe_sem, 1)

        @block.vector
        def _(vector):
            vector.wait_ge(pe_sem, 1)
            # compact the two diagonal 64x64 blocks of each class image:
            # rows 0:64 take columns j*128+c, rows 64:128 take j*128+64+c
            src_a = ps[0:P, :NCLASS * CHUNK].rearrange(
                "p (j c) -> p j c", c=CHUNK)[:, :, 0:P]
            src_b = ps[P:CHUNK, :NCLASS * CHUNK].rearrange(
                "p (j c) -> p j c", c=CHUNK)[:, :, P:CHUNK]
            dst_a = r[0:P, :].rearrange("p (j c) -> p j c", c=P)
            dst_b = r[P:CHUNK, :].rearrange("p (j c) -> p j c", c=P)
            nc.vector.tensor_copy(dst_a, src_a)
            nc.vector.tensor_copy(dst_b, src_b).then_inc(dve_sem, 1)

    nc.compile()
    return nc, {"C": C}


def _pack_shards(embed, targets):
    """Sort by class, split per class across cores, zero-pad to an even
    number of class-pure 128-row chunks per core, pack partition-major."""
    m = embed.shape[0]
    t = np.asarray(targets).astype(np.int64).ravel()
    counts = np.bincount(t, minlength=NCLASS).astype(np.int64)
    order = np.argsort(t, kind="stable")
    se = np.ascontiguousarray(np.asarray(embed, dtype=np.float32)[order])

    # even chunk count per class per device
    chunks_dev = 2 * np.maximum(1, -(-counts // (NCORES * 2 * CHUNK))).astype(int)
    C = int(chunks_dev.sum())
    X = np.zeros((NCORES, C * CHUNK, P), dtype=np.float32)
    cls_ofs = np.concatenate([[0], np.cumsum(counts)])
    row0 = np.concatenate([[0], np.cumsum(chunks_dev * CHUNK)])
    for j in range(NCLASS):
        cj = int(counts[j])
        base, rem = divmod(cj, NCORES)
        sizes = base + (np.arange(NCORES) < rem)
        starts = cls_ofs[j] + np.concatenate([[0], np.cumsum(sizes)[:-1]])
        assert sizes.max() <= chunks_dev[j] * CHUNK
        for d in range(NCORES):
            X[d, row0[j]:row0[j] + sizes[d]] = se[starts[d]:starts[d] + sizes[d]]

    Xc = X.astype(_NP_DT[COMPUTE_DTYPE])
    packed = np.ascontiguousarray(
        Xc.reshape(NCORES, C, CHUNK, P).transpose(0, 2, 1, 3)
        .reshape(NCORES, CHUNK, C * P))
    return packed, counts, tuple(int(v) for v in chunks_dev), m


def _ensure_ntff_hook():
    """The agent image's antenv lacks axon_hooks; synthesize it and register
    the ctypes NTFF profile hook so run_bass_kernel_spmd(trace=True) works."""
    import sys, types
    try:
        import antenv.axon_hooks  # noqa: F401
        return True
    except ImportError:
        pass
    try:
        import antenv
        from trn_agent_boot.trn_boot import _ntff_profile_via_ctypes
        mod = types.ModuleType("antenv.axon_hooks")
        _hook = [None]
        mod.set_axon_ntff_profile_hook = lambda h: _hook.__setitem__(0, h)
        mod.get_axon_ntff_profile_hook = lambda: _hook[0]
        sys.modules["antenv.axon_hooks"] = mod
        antenv.axon_hooks = mod
        mod.set_axon_ntff_profile_hook(
            _ntff_profile_via_ctypes("/opt/axon/libaxon_pjrt.so"))
        return True
    except Exception:
        return False


def kernel(embed, targets):
    global LAST_EXEC_NS, LAST_RESULTS
    packed, counts, chunks_dev, m = _pack_shards(embed, targets)

    key = (chunks_dev, COMPUTE_DTYPE, NWARM)
    if key not in _prog_cache:
        _prog_cache[key] = _build_program(chunks_dev, COMPUTE_DTYPE)
    nc, meta = _prog_cache[key]

    from concourse.bass_utils import run_bass_kernel_spmd
    in_maps = [{"x": packed[d]} for d in range(NCORES)]
    do_trace = bool(PROFILE) and _ensure_ntff_hook()
    res = run_bass_kernel_spmd(nc, in_maps, core_ids=list(range(NCORES)),
                               trace=do_trace)
    LAST_EXEC_NS = res.exec_time_ns
    LAST_RESULTS = res

    # host reduction: per-class Gram = sum over cores of the two 64x64 blocks
    grams = np.zeros((NCLASS, P, P), dtype=np.float64)
    for r in res.results:
        o = np.asarray(r["out"], dtype=np.float64)
        for j in range(NCLASS):
            grams[j] += o[:P, j * P:(j + 1) * P] + o[P:, j * P:(j + 1) * P]

    eye = np.eye(P, dtype=np.float64)
    g_tot = grams.sum(axis=0)
    ld_tot = np.linalg.slogdet(eye + GAM1 * (P / (m * EPS)) * g_tot)[1]
    tr_pi = counts.astype(np.float64) + 1e-8
    compress = 0.0
    for j in range(NCLASS):
        ldj = np.linalg.slogdet(eye + (P / (tr_pi[j] * EPS)) * grams[j])[1]
        compress += ldj * tr_pi[j] / m / 2.0
    loss = GAM2 * (-ld_tot / 2.0) + compress
    return np.array(loss, dtype=np.float32)
